# revision 1
# baseline (speedup 1.0000x reference)
"""Gumbel-softmax hard sampling (B=4096, C=32000, f32) on 8 trn2 NeuronCores.

Math: output = one_hot(argmax(softmax((logits+g)/tau))) with tau=1 and
g = -log(EPS - log(u+EPS)).  Softmax is strictly monotonic per row, so
argmax(softmax(s)) == argmax(s): we only need argmax(logits + g) and a
one-hot materialization — no exp/softmax on device.

Sharding: batch rows split 8 ways (512 rows/core, pure data parallel).

v2 design (vs the 648us baseline): the one-hot output is 1-sparse, and
run_bass_kernel_spmd's documented contract pre-zeroes ExternalOutput
buffers (natively via np.zeros out_maps; under axon via donated zeroed
PJRT buffers — see bass2jax.run_bass_via_pjrt).  So the device writes
ONLY the 512 ones via an indirect (scatter) DMA instead of streaming
the dense 65.5 MB/core one-hot.  That removes a third of the DMA
traffic and the entire ACT pass-2, leaving:

  per core (512 x 32000), 4 row-blocks x 8 col-tiles of [128, 4000]:
    ACT : t1 = Ln(u + EPS); t2 = Ln(EPS - t1)          (2 LUT passes)
    POOL: score = logits - t2 (gpsimd streaming sub, in-place over the
          logits tile) -- sheds one full DVE scan; DVE's 1x-mode ops
          never contend with gpsimd for the shared SBUF port
    DVE : max        top-8 values per tile (sorted descending)
    DVE : max_index  their 8 column indices in one more 4000-wide pass
    cross-tile argmax via is_equal indicator + reduce_sum (tiny ops)
    POOL: indirect_dma_start scatters 1.0f per row into the flat
          [ROWS*C] one-hot at row*C + argmax.

  DMA rings: logits + half of u on the SP HWDGE ring, other half of u
  on the ACT HWDGE ring (each ring serializes its own transfers at
  ~330 GB/s; HBM caps the aggregate at ~358 GB/s/core).

  roofline: DMA loads 131 MB/core @ ~358 GB/s = ~366 us;
            DVE ~280 us; POOL ~260 us; ACT ~226 us  ->  DMA-bound.

Host patch: the ACT Ln LUT differs from XLA's log by ~ulps, which can
flip rows whose top-2 scores are within that noise.  The kernel also
returns the 64 candidate indices/row (top-8 of each 4000-wide tile);
the host recomputes those candidates' scores with jax f32 (bitwise-
matching the reference expression) and patches any rows where the
winner differs.  Candidate sets always contain the true argmax (top-8
containment would need 8 same-tile scores within LUT noise of the
max), so the patched output is exact.
"""

import sys

if "/opt/trn_rl_repo" not in sys.path:
    sys.path.insert(0, "/opt/trn_rl_repo")

from contextlib import ExitStack  # noqa: E402

import numpy as np  # noqa: E402

import concourse.bass as bass  # noqa: E402
import concourse.tile as tile  # noqa: E402
from concourse import bacc, mybir  # noqa: E402
from concourse.bass_utils import run_bass_kernel_spmd  # noqa: E402

EPS = 1e-10
B, C = 4096, 32000
N_CORES = 8
ROWS = B // N_CORES          # 512 rows per core
P = 128                      # partitions per row-block
N_BLOCKS = ROWS // P         # 4
W = 4000                     # col-tile width
T = C // W                   # 8 col-tiles
NCAND = T * 8                # 64 candidates per row (top-8 per tile)
F32 = mybir.dt.float32
U16 = mybir.dt.uint16
I32 = mybir.dt.int32


def _setup(nc, onehot_kind="ExternalOutput"):
    # EPS as a per-partition const AP for activation bias
    eps_t = nc.alloc_sbuf_tensor("const-eps", [128, 1], F32)
    nc.gpsimd.memset(eps_t.ap(), EPS)
    nc.const_aps.aps[(F32, EPS)] = eps_t.ap()
    nc.all_engine_barrier()
    aps = dict(
        logits=nc.dram_tensor("logits", [ROWS, C], F32, kind="ExternalInput").ap(),
        u=nc.dram_tensor("u", [ROWS, C], F32, kind="ExternalInput").ap(),
        # flat so the scatter can index single elements on axis 0
        onehot=nc.dram_tensor("onehot", [ROWS * C, 1], F32, kind=onehot_kind).ap(),
        chosen=nc.dram_tensor("chosen", [ROWS], F32, kind="ExternalOutput").ap(),
        cand=nc.dram_tensor("cand", [ROWS, NCAND], F32, kind="ExternalOutput").ap(),
    )
    return aps


U_ENG = "sp"         # "split" (SP/ACT alternating), "sp", "pool"
LBUFS = 5
UBUFS = 4


def _emit_blocks(nc, tc, ctx, aps, block_list, mode="full"):
    logits, u = aps["logits"], aps["u"]
    onehot, chosen_d, cand_d = aps["onehot"], aps["chosen"], aps["cand"]
    lpool = ctx.enter_context(tc.tile_pool(name="lpool", bufs=LBUFS))
    upool = ctx.enter_context(tc.tile_pool(name="upool", bufs=UBUFS))
    cpool = ctx.enter_context(tc.tile_pool(name="cpool", bufs=1))
    spool = ctx.enter_context(tc.tile_pool(name="spool", bufs=2))

    # constants: per-(partition,block) flat row base (row*C) and a ones column
    # (iota pattern steps must fit int16, so build row = p + j*P first and
    # scale by C in f32 — max value 16.38M < 2^24 stays exact)
    rowbase_i = cpool.tile([P, N_BLOCKS], I32)
    nc.gpsimd.iota(
        rowbase_i[:], pattern=[[P, N_BLOCKS]], base=0, channel_multiplier=1
    )
    rowbase_f = cpool.tile([P, N_BLOCKS], F32)
    nc.vector.tensor_copy(rowbase_f[:], rowbase_i[:])
    nc.vector.tensor_scalar_mul(rowbase_f[:], rowbase_f[:], float(C))
    ones_t = cpool.tile([P, 1], F32)
    nc.gpsimd.memset(ones_t[:], 1.0)

    def emit_block(b):
        r0 = b * P
        if mode not in ("dma", "nodve", "actonly", "subonly"):
            s8all = spool.tile([P, NCAND], F32, tag="s8all")  # top-8 per tile
            mi = spool.tile([P, NCAND], U16, tag="mi")        # local indices
        for t in range(T):
            c0 = t * W
            # DMA ring split: logits on the SP HWDGE ring, u alternating
            # between SP and ACT rings.  The subtract runs on GPSIMD (the
            # DVE/ACT engines carry the per-element scans), so no ring ever
            # exceeds ~300us and DVE keeps only 2 scans/tile.
            lt = lpool.tile([P, W], F32)
            nc.sync.dma_start(lt[:], logits[r0 : r0 + P, c0 : c0 + W])
            ut = upool.tile([P, W], F32)
            if U_ENG == "sp":
                u_eng = nc.sync
            elif U_ENG == "pool":
                u_eng = nc.gpsimd
            else:
                u_eng = nc.sync if t % 2 == 0 else nc.scalar
            u_eng.dma_start(ut[:], u[r0 : r0 + P, c0 : c0 + W])
            if mode == "dma":
                continue
            if mode == "subonly":
                nc.gpsimd.tensor_sub(lt[:], lt[:], ut[:])
                continue
            if mode == "dvescan":
                nc.vector.max(s8all[:, t * 8 : t * 8 + 8], lt[:])
                nc.vector.max_index(
                    mi[:, t * 8 : t * 8 + 8], s8all[:, t * 8 : t * 8 + 8], lt[:]
                )
                continue
            # t1 = ln(u + eps); t2 = ln(eps - t1)   (in-place on ut)
            nc.scalar.activation(
                ut[:], ut[:], mybir.ActivationFunctionType.Ln, bias=EPS, scale=1.0
            )
            nc.scalar.activation(
                ut[:], ut[:], mybir.ActivationFunctionType.Ln, bias=EPS, scale=-1.0
            )
            if mode == "actonly":
                continue
            # score = logits - t2 (in-place on lt) on GPSIMD (Q7 streaming;
            # DVE 1x-mode ops never contend for the shared SBUF port)
            if mode == "nosub":
                nc.vector.tensor_sub(lt[:], lt[:], ut[:])
            else:
                nc.gpsimd.tensor_sub(lt[:], lt[:], ut[:])
            if mode == "nodve":
                continue
            # top-8 values per tile (sorted desc), then their tile-local
            # column indices in one more pass
            nc.vector.max(s8all[:, t * 8 : t * 8 + 8], lt[:])
            nc.vector.max_index(
                mi[:, t * 8 : t * 8 + 8], s8all[:, t * 8 : t * 8 + 8], lt[:]
            )

        if mode != "full" and mode != "nosub":
            return
        # global candidate indices (f32): cif = f32(mi) + 4000*t per group
        cif = spool.tile([P, NCAND], F32, tag="cif")
        nc.vector.tensor_copy(cif[:], mi[:])
        for t in range(1, T):
            nc.vector.tensor_scalar_add(
                cif[:, t * 8 : t * 8 + 8], cif[:, t * 8 : t * 8 + 8], float(W * t)
            )
        # cross-tile argmax: chosen = sum(cif * (s8all == max(s8all)))
        gm8 = spool.tile([P, 8], F32, tag="gm8")
        nc.vector.max(gm8[:], s8all[:])
        eq = spool.tile([P, NCAND], F32, tag="eq")
        nc.vector.tensor_scalar(
            eq[:], s8all[:], gm8[:, 0:1], None, op0=mybir.AluOpType.is_equal
        )
        nc.vector.tensor_mul(eq[:], eq[:], cif[:])
        chosen = spool.tile([P, 1], F32, tag="chosen")
        nc.vector.reduce_sum(chosen[:], eq[:], axis=mybir.AxisListType.X)
        # clamp: duplicated maxima make chosen a sum of indices; keep the
        # scatter in-bounds (host patches such rows from `cand` anyway)
        nc.vector.tensor_scalar_min(chosen[:], chosen[:], float(C - 1))

        nc.scalar.dma_start(chosen_d[r0 : r0 + P], chosen[:])
        nc.scalar.dma_start(cand_d[r0 : r0 + P, :], cif[:])

        # scatter 1.0f at flat index row*C + chosen (output is pre-zeroed)
        flatf = spool.tile([P, 1], F32, tag="flatf")
        nc.vector.tensor_tensor(
            flatf[:], chosen[:], rowbase_f[:, b : b + 1], op=mybir.AluOpType.add
        )
        flati = spool.tile([P, 1], I32, tag="flati")
        nc.vector.tensor_copy(flati[:], flatf[:])
        nc.gpsimd.indirect_dma_start(
            out=onehot[:],
            out_offset=bass.IndirectOffsetOnAxis(ap=flati[:, :1], axis=0),
            in_=ones_t[:],
            in_offset=None,
        )

    def one_rep():
        for b in block_list:
            emit_block(b)

    return one_rep


def build_program(reps: int = 1):
    nc = bacc.Bacc(
        "TRN2", target_bir_lowering=False, debug=False, num_devices=N_CORES
    )
    aps = _setup(nc)
    with tile.TileContext(nc) as tc, ExitStack() as ctx:
        one_rep = _emit_blocks(nc, tc, ctx, aps, list(range(N_BLOCKS)))
        for _ in range(reps):
            one_rep()
    nc.compile()
    return nc


def build_loop_program(loops: int = 64, mode: str = "full"):
    """Benchmark-only variant: device-side For_i loop around the full body.

    The one-hot DRAM tensor is Internal here (scatter still writes it, but
    it is not an output), so per-dispatch host<->device traffic is only the
    small chosen/cand outputs — the inputs stay resident on device across
    timed calls (see bench.py).
    """
    nc = bacc.Bacc(
        "TRN2", target_bir_lowering=False, debug=False, num_devices=N_CORES
    )
    aps = _setup(nc, onehot_kind="Internal")
    with tile.TileContext(nc) as tc, ExitStack() as ctx:
        one_rep = _emit_blocks(nc, tc, ctx, aps, list(range(N_BLOCKS)), mode=mode)
        with tc.For_i(0, loops, 1):
            one_rep()
    nc.compile()
    return nc


_NC_CACHE = None


def _get_program():
    global _NC_CACHE
    if _NC_CACHE is None:
        _NC_CACHE = build_program()
    return _NC_CACHE


def _host_refine(logits, u, cand_idx):
    """Recompute candidate scores with jax f32 (matches reference bitwise);
    return the reference-semantics argmax per row (first occurrence)."""
    import jax
    import jax.numpy as jnp

    rows = np.arange(B)[:, None]
    lg = logits[rows, cand_idx]
    ug = u[rows, cand_idx]
    with jax.default_device(jax.local_devices(backend="cpu")[0]):
        g = -jnp.log(EPS - jnp.log(jnp.asarray(ug) + EPS))
        sc = np.asarray(jnp.asarray(lg) + g)
    m = sc.max(axis=1, keepdims=True)
    masked = np.where(sc == m, cand_idx, np.iinfo(np.int64).max)
    return masked.min(axis=1)


def kernel(logits: np.ndarray, u: np.ndarray, **_) -> np.ndarray:
    logits = np.ascontiguousarray(logits, dtype=np.float32)
    u = np.ascontiguousarray(u, dtype=np.float32)
    nc = _get_program()
    in_maps = [
        {
            "logits": logits[i * ROWS : (i + 1) * ROWS],
            "u": u[i * ROWS : (i + 1) * ROWS],
        }
        for i in range(N_CORES)
    ]
    res = run_bass_kernel_spmd(nc, in_maps, core_ids=list(range(N_CORES)))
    out = np.concatenate(
        [r["onehot"].reshape(ROWS, C) for r in res.results], axis=0
    )
    chosen = np.concatenate([r["chosen"] for r in res.results]).reshape(B)
    cand = np.concatenate([r["cand"] for r in res.results]).reshape(B, NCAND)

    chosen_i = np.rint(chosen).astype(np.int64)
    cand_i = np.clip(np.rint(cand).astype(np.int64), 0, C - 1)
    host_idx = _host_refine(logits, u, cand_i)

    bad = np.nonzero(host_idx != chosen_i)[0]
    for r in bad:
        ci = chosen_i[r]
        if 0 <= ci < C:
            out[r, ci] = 0.0
        out[r, host_idx[r]] = 1.0
    return out



# revision 2
# speedup vs baseline: 1.4172x; 1.4172x over previous
"""Gumbel-softmax hard sampling (B=4096, C=32000, f32) on 8 trn2 NeuronCores.

Math: output = one_hot(argmax(softmax((logits+g)/tau))) with tau=1 and
g = -log(EPS - log(u+EPS)).  Softmax is strictly monotonic per row, so
argmax(softmax(s)) == argmax(s): we only need argmax(logits + g) and a
one-hot materialization — no exp/softmax on device.

Sharding: batch rows split 8 ways (512 rows/core, pure data parallel).

v3 design (vs the 457us v2): the device streams logits+u (131 MB/core,
the roofline term) and reduces each row to 64 candidate indices — the
top-8 of each 4000-wide column tile — via DVE max/max_index scans.
That candidate list is the ONLY device output (u16 [512, 64]/core).
v2 additionally computed the cross-tile argmax and scattered the 1.0s
on device, but the host must rescore all candidates anyway (the ACT Ln
LUT differs from XLA's log by ~ulps, which can flip near-tie rows), so
the device argmax/scatter chain (~10 dependent small DVE ops + an
indirect DMA per 128-row block) was pure overhead: ablation shows it
cost ~87us/rep of pipeline serialization on top of the 377us pure-DMA
floor.  With it gone, every engine queue is a clean per-tile stream:

  per core (512 x 32000), 4 row-blocks x 8 col-tiles of [128, 4000]:
    ACT : t1 = Ln(u + EPS); t2 = Ln(EPS - t1)          (2 LUT passes)
    POOL: score = logits - t2 (gpsimd streaming sub, in-place over the
          logits tile)
    DVE : max        top-8 values per tile (sorted descending)
    DVE : max_index  their 8 column indices in one more 4000-wide pass
    ACT ring: one [128, 64] u16 DMA out per block (the indices)

  roofline: DMA loads 131 MB/core; pure-DMA ablation of this exact
  tiling measures ~377us/rep -> DMA-bound target.

Host side: global candidate indices = mi + 4000*tile; the host gathers
logits/u at the 64 candidates per row, recomputes scores with jax f32
(bitwise-matching the reference expression), takes the first-occurrence
argmax, and writes the one-hot directly into the zeroed output array.
Candidate sets always contain the true argmax (missing it would need 8
same-tile scores within LUT noise of the max), so the result is exact.
"""

import sys

if "/opt/trn_rl_repo" not in sys.path:
    sys.path.insert(0, "/opt/trn_rl_repo")

from contextlib import ExitStack  # noqa: E402

import numpy as np  # noqa: E402

import concourse.tile as tile  # noqa: E402
from concourse import bacc, mybir  # noqa: E402
from concourse.bass_utils import run_bass_kernel_spmd  # noqa: E402

EPS = 1e-10
B, C = 4096, 32000
N_CORES = 8
ROWS = B // N_CORES          # 512 rows per core
P = 128                      # partitions per row-block
N_BLOCKS = ROWS // P         # 4
W = 4000                     # col-tile width
T = C // W                   # 8 col-tiles
NCAND = T * 8                # 64 candidates per row (top-8 per tile)
F32 = mybir.dt.float32
U16 = mybir.dt.uint16


def _setup(nc):
    # EPS as a per-partition const AP for activation bias
    eps_t = nc.alloc_sbuf_tensor("const-eps", [128, 1], F32)
    nc.gpsimd.memset(eps_t.ap(), EPS)
    nc.const_aps.aps[(F32, EPS)] = eps_t.ap()
    nc.all_engine_barrier()
    aps = dict(
        logits=nc.dram_tensor("logits", [ROWS, C], F32, kind="ExternalInput").ap(),
        u=nc.dram_tensor("u", [ROWS, C], F32, kind="ExternalInput").ap(),
        mi=nc.dram_tensor("mi", [ROWS, NCAND], U16, kind="ExternalOutput").ap(),
    )
    return aps


U_ENG = "sp"         # "split" (SP/ACT alternating), "sp", "pool"
LBUFS = 5
UBUFS = 4


def _emit_blocks(nc, tc, ctx, aps, block_list, mode="full"):
    logits, u = aps["logits"], aps["u"]
    mi_d = aps["mi"]
    lpool = ctx.enter_context(tc.tile_pool(name="lpool", bufs=LBUFS))
    upool = ctx.enter_context(tc.tile_pool(name="upool", bufs=UBUFS))
    spool = ctx.enter_context(tc.tile_pool(name="spool", bufs=2))

    def emit_block(b):
        r0 = b * P
        if mode not in ("dma", "nodve", "actonly", "subonly"):
            s8all = spool.tile([P, NCAND], F32, tag="s8all")  # top-8 per tile
            mi = spool.tile([P, NCAND], U16, tag="mi")        # local indices
        for t in range(T):
            c0 = t * W
            # DMA ring split: logits on the SP HWDGE ring, u per U_ENG.
            lt = lpool.tile([P, W], F32)
            nc.sync.dma_start(lt[:], logits[r0 : r0 + P, c0 : c0 + W])
            ut = upool.tile([P, W], F32)
            if U_ENG == "sp":
                u_eng = nc.sync
            elif U_ENG == "pool":
                u_eng = nc.gpsimd
            else:
                u_eng = nc.sync if t % 2 == 0 else nc.scalar
            u_eng.dma_start(ut[:], u[r0 : r0 + P, c0 : c0 + W])
            if mode == "dma":
                continue
            if mode == "subonly":
                nc.gpsimd.tensor_sub(lt[:], lt[:], ut[:])
                continue
            if mode == "dvescan":
                nc.vector.max(s8all[:, t * 8 : t * 8 + 8], lt[:])
                nc.vector.max_index(
                    mi[:, t * 8 : t * 8 + 8], s8all[:, t * 8 : t * 8 + 8], lt[:]
                )
                continue
            # t1 = ln(u + eps); t2 = ln(eps - t1)   (in-place on ut)
            nc.scalar.activation(
                ut[:], ut[:], mybir.ActivationFunctionType.Ln, bias=EPS, scale=1.0
            )
            nc.scalar.activation(
                ut[:], ut[:], mybir.ActivationFunctionType.Ln, bias=EPS, scale=-1.0
            )
            if mode == "actonly":
                continue
            # score = logits - t2 (in-place on lt) on GPSIMD (Q7 streaming;
            # DVE 1x-mode ops never contend for the shared SBUF port)
            if mode == "nosub":
                nc.vector.tensor_sub(lt[:], lt[:], ut[:])
            else:
                nc.gpsimd.tensor_sub(lt[:], lt[:], ut[:])
            if mode == "nodve":
                continue
            # top-8 values per tile (sorted desc), then their tile-local
            # column indices in one more pass
            nc.vector.max(s8all[:, t * 8 : t * 8 + 8], lt[:])
            nc.vector.max_index(
                mi[:, t * 8 : t * 8 + 8], s8all[:, t * 8 : t * 8 + 8], lt[:]
            )

        if mode not in ("full", "nosub", "dvescan"):
            return
        # candidate indices out; the host does the argmax + one-hot
        nc.scalar.dma_start(mi_d[r0 : r0 + P, :], mi[:])

    def one_rep():
        for b in block_list:
            emit_block(b)

    return one_rep


def build_program(reps: int = 1, mode: str = "full"):
    nc = bacc.Bacc(
        "TRN2", target_bir_lowering=False, debug=False, num_devices=N_CORES
    )
    aps = _setup(nc)
    with tile.TileContext(nc) as tc, ExitStack() as ctx:
        one_rep = _emit_blocks(nc, tc, ctx, aps, list(range(N_BLOCKS)), mode=mode)
        for _ in range(reps):
            one_rep()
    nc.compile()
    return nc


def build_loop_program(loops: int = 64, mode: str = "full"):
    """Benchmark variant: device-side For_i loop around the full body."""
    nc = bacc.Bacc(
        "TRN2", target_bir_lowering=False, debug=False, num_devices=N_CORES
    )
    aps = _setup(nc)
    with tile.TileContext(nc) as tc, ExitStack() as ctx:
        one_rep = _emit_blocks(nc, tc, ctx, aps, list(range(N_BLOCKS)), mode=mode)
        with tc.For_i(0, loops, 1):
            one_rep()
    nc.compile()
    return nc


_NC_CACHE = None


def _get_program():
    global _NC_CACHE
    if _NC_CACHE is None:
        _NC_CACHE = build_program()
    return _NC_CACHE


def _host_refine(logits, u, cand_idx):
    """Recompute candidate scores with jax f32 (matches reference bitwise);
    return the reference-semantics argmax per row (first occurrence)."""
    import jax
    import jax.numpy as jnp

    rows = np.arange(B)[:, None]
    lg = logits[rows, cand_idx]
    ug = u[rows, cand_idx]
    with jax.default_device(jax.local_devices(backend="cpu")[0]):
        g = -jnp.log(EPS - jnp.log(jnp.asarray(ug) + EPS))
        sc = np.asarray(jnp.asarray(lg) + g)
    m = sc.max(axis=1, keepdims=True)
    masked = np.where(sc == m, cand_idx, np.iinfo(np.int64).max)
    return masked.min(axis=1)


def kernel(logits: np.ndarray, u: np.ndarray, **_) -> np.ndarray:
    logits = np.ascontiguousarray(logits, dtype=np.float32)
    u = np.ascontiguousarray(u, dtype=np.float32)
    nc = _get_program()
    in_maps = [
        {
            "logits": logits[i * ROWS : (i + 1) * ROWS],
            "u": u[i * ROWS : (i + 1) * ROWS],
        }
        for i in range(N_CORES)
    ]
    res = run_bass_kernel_spmd(nc, in_maps, core_ids=list(range(N_CORES)))
    mi = np.concatenate([r["mi"] for r in res.results]).reshape(B, NCAND)

    offs = (np.arange(T, dtype=np.int64) * W).repeat(8)      # [64]
    cand = mi.astype(np.int64) + offs[None, :]
    idx = _host_refine(logits, u, cand)

    out = np.zeros((B, C), dtype=np.float32)
    out[np.arange(B), idx] = 1.0
    return out
